# revision 34
# baseline (speedup 1.0000x reference)
"""Trainium2 Bass kernel for nn_BasicBlock_88665304858673 (spiking BasicBlock).

Computation (dead code removed -- mem2/o2/m2, memd/od and inp_u never reach
the outputs):

  per time step t (T=4):
    I1_t   = conv1(x_t)            3x3 stride2 pad1, 256->512, BN-folded
    mem1  += I1_t ; o1_t = (mem1 >= vth1) ; mem1 -= o1_t*vth1 ; mask1 |= o1_t
    out_s_t = conv2(o1_t) + convd(x_t)     (3x3 s1 p1 and 1x1 s2)
    memf  += out_s_t ; o3_t = (memf >= vth_if) ; memf -= o3_t*vth_if ; mask3 |= o3_t
  outputs: o3_3, out_s_3, and the ANN branch
    a     = relu(conv1(inp_c)) * mask1
    out_c = relu(conv2(a) + convd(inp_c)) * mask3

Sharding: data-parallel over batch B=32 -> 8 cores x 4 images (2 pairs of 2;
matmul moving dim N = 2*196 = 392).

Matmul dtype plan (fp32 runs at 4 cycles/col on the PE; bf16 and f32r --
fp32 bits rounded to 11 mantissa bits by the PE -- run at 1 cycle/col):
  conv1  : bf16 3-term split  xh*wh + xh*wl + xl*wh   (~2^-17 accurate; conv1
           feeds the mem1 threshold whose spike flips amplify through conv2,
           so single-rounded terms are NOT enough -- measured in simulation)
  conv2  : f32r single term. Input o1 is binary {0,1} => exact in any dtype;
           only the weight sees the 2^-12 rounding, and errors here only
           shift memf slightly (simulated combined rel err ~7e-3 vs 2e-2 gate)
  convd  : f32r single term (same argument, small magnitude)
  ANN    : conv1 bf16 single (reuses resident w1-hi), conv2/convd f32r
           (reuse resident f32r weights); no thresholds downstream.

All matmuls are dense [128, 392]: inputs are host-side phase-decomposed into
the four stride-2 parity planes of the zero-padded 30x30 image (sizes 15x15,
15x14, 14x15, 14x14 concat -> 842), so each conv1 tap reads a stride-1 14x14
window and writes the full psum tile. Zero padding makes every tap valid
everywhere and keeps innermost runs even (an f32r moving-operand requirement).

o1 spikes are stored as 3 kx-pre-shifted zero-padded copies so every conv2
tap reads two fully CONTIGUOUS 196-element runs -- strided f32r moving
operands cost ~2 PE cycles per run transition (measured 202ns vs 180ns per
392-col matmul).

Weights stay resident in SBUF for the whole kernel (loaded once, cok-major
so per-cok DMA chunks unlock the first conv groups early): w1 hi/lo bf16,
w2/wd f32r. Measured: 1266927ns (fp32 baseline) -> 627352ns, ~97% tensor
engine occupancy, rel err 1.18e-2 (gate 2e-2, bit-stable across runs).
"""

import numpy as np
import ml_dtypes

EPS = 1e-5
NCORES = 8
BPC = 4          # images per core
NPAIR = 2        # image pairs per core
NIMG = 2         # images per pair
PIX = 196
NN = NIMG * PIX  # moving dim: 392

_CACHE = {}
TRACE = False
LAST_RESULT = None

# parity plane (rp, cp) -> (offset, nrows, ncols) within the per-image layout.
# Row widths are padded even (16/14/16/14) and the image stride (872) is even
# so every bf16 row start lands 4-byte aligned.
_PLANE = {
    (0, 0): (0, 15, 15),
    (0, 1): (225, 15, 14),
    (1, 0): (435, 14, 15),
    (1, 1): (645, 14, 14),
}
PLN = 842      # xl layout: 4 parity planes + tail pad
PLNH = 1248    # xh/xc layout: planes + 14-wide kx=2 duplicates at 842/1052
# rp -> (dup offset, nrows); dup[r, c] = plane(rp,0)[r, c+1], rows 14-wide so
# every run start is even and a 14-row read is fully contiguous
_DUP = {0: (842, 15), 1: (1052, 14)}


def _tap_phase(k):
    """3x3 tap index along one axis -> (parity, start offset in plane)."""
    return (1, 0) if k == 1 else (0, 1 if k == 2 else 0)


def _build(cfg):
    """cfg = (bias1_any, bias2_any, vth1_scalar_or_None, vthf_scalar_or_None)"""
    import concourse.bacc as bacc
    import concourse.mybir as mybir
    import concourse.tile as tile

    F32 = mybir.dt.float32
    F32R = mybir.dt.float32r
    BF16 = mybir.dt.bfloat16
    Alu = mybir.AluOpType
    Act = mybir.ActivationFunctionType
    bias1_any, bias2_any, vth1_c, vthf_c = cfg

    nc = bacc.Bacc(None, target_bir_lowering=False)

    W1Hd = nc.dram_tensor("W1H", [128, 2 * 9 * 512], BF16, kind="ExternalInput")
    W1Ld = nc.dram_tensor("W1L", [128, 2 * 9 * 512], BF16, kind="ExternalInput")
    W2Rd = nc.dram_tensor("W2R", [128, 4 * 9 * 512], F32R, kind="ExternalInput")
    WDRd = nc.dram_tensor("WDR", [128, 2 * 512], F32R, kind="ExternalInput")
    XSHd = nc.dram_tensor("XSH", [NPAIR, 4, 2, 128, NIMG * PLNH], BF16,
                          kind="ExternalInput")
    XSLd = nc.dram_tensor("XSL", [NPAIR, 4, 2, 128, NIMG * PLN], BF16,
                          kind="ExternalInput")
    XDd = nc.dram_tensor("XD", [NPAIR, 4, 2, 128, NIMG * 196], F32R,
                         kind="ExternalInput")
    XCd = nc.dram_tensor("XC", [NPAIR, 2, 128, NIMG * PLNH], BF16,
                         kind="ExternalInput")
    XCDd = nc.dram_tensor("XCD", [NPAIR, 2, 128, NIMG * 196], F32R,
                          kind="ExternalInput")
    ZPADd = nc.dram_tensor("ZPAD", [128, 1288], F32R, kind="ExternalInput")
    if bias1_any:
        B1Hd = nc.dram_tensor("B1H", [1, 512], BF16, kind="ExternalInput")
        B1Ld = nc.dram_tensor("B1L", [1, 512], BF16, kind="ExternalInput")
    if bias2_any:
        B2Hd = nc.dram_tensor("B2H", [1, 512], F32R, kind="ExternalInput")
        B2Ld = nc.dram_tensor("B2L", [1, 512], F32R, kind="ExternalInput")
        ONERd = nc.dram_tensor("ONER", [1, NN], F32R, kind="ExternalInput")
    if vth1_c is None:
        V1d = nc.dram_tensor("VTH1R", [128, 4 * NN], F32, kind="ExternalInput")
    if vthf_c is None:
        VFd = nc.dram_tensor("VTHFR", [128, 4 * NN], F32, kind="ExternalInput")
    O3d = nc.dram_tensor("O3", [NPAIR, 128, 4 * NN], F32, kind="ExternalOutput")
    IUd = nc.dram_tensor("IU", [NPAIR, 4, 128, NN], F32, kind="ExternalOutput")
    OCd = nc.dram_tensor("OC", [NPAIR, 128, 4 * NN], F32, kind="ExternalOutput")

    with tile.TileContext(nc) as tc:
        with tc.tile_pool(name="wpool", bufs=1) as wp, \
             tc.tile_pool(name="xpool", bufs=2) as xp, \
             tc.tile_pool(name="spool", bufs=1) as st, \
             tc.tile_pool(name="ostr", bufs=1) as ostr, \
             tc.tile_pool(name="xdpool", bufs=1) as xdp, \
             tc.tile_pool(name="iupool", bufs=1) as iup, \
             tc.tile_pool(name="pspool", bufs=4, space="PSUM") as pp:

            w1h = wp.tile([128, 2 * 9 * 512], BF16, name="w1h")
            w1l = wp.tile([128, 2 * 9 * 512], BF16, name="w1l")
            w2r = wp.tile([128, 4 * 9 * 512], F32R, name="w2r")
            wdr = wp.tile([128, 2 * 512], F32R, name="wdr")

            mem1 = [st.tile([128, NN], F32, name=f"mem1_{k}") for k in range(4)]
            memf = [st.tile([128, NN], F32, name=f"memf_{k}") for k in range(4)]
            U8 = mybir.dt.uint8
            mask1 = [st.tile([128, NN], U8, name=f"mask1_{k}") for k in range(4)]
            mask3 = [st.tile([128, NN], U8, name=f"mask3_{k}") for k in range(4)]
            # o1 spikes stored as 3 kx-pre-shifted copies (one per conv2
            # column tap), each [2 img, 15 rows, 14 cols] zero-padded, plus a
            # 28-elem zero tail: copy kx at [kx*420 + b*210 + y*14 + x] holds
            # o1[b, y-1, x-1+kx].  A tap (ky,kx) then reads TWO CONTIGUOUS
            # 196-elem runs (one per image) -- f32r matmuls pay ~2 cycles per
            # rhs run transition, so 28 short runs cost ~24ns/matmul extra.
            # Vertical overruns (row 15) land on the next block's zero pad row.
            # Double-buffered by timestep parity; parity-0 slots are reused
            # for the ANN gated-relu input.
            o1p = [[st.tile([128, 1288], F32R, name=f"o1p_{k}_{par}")
                    for par in range(2)] for k in range(4)]


            if bias1_any:
                ones_b = st.tile([1, NN], BF16, name="ones_b")
                nc.vector.memset(ones_b[:], 1.0)
                b1h = st.tile([1, 512], BF16, name="b1h")
                nc.sync.dma_start(out=b1h[:], in_=B1Hd[:])
                b1l = st.tile([1, 512], BF16, name="b1l")
                nc.sync.dma_start(out=b1l[:], in_=B1Ld[:])
            if bias2_any:
                ones_r = st.tile([1, NN], F32R, name="ones_r")
                nc.sync.dma_start(out=ones_r[:], in_=ONERd[:])
                b2h = st.tile([1, 512], F32R, name="b2h")
                nc.sync.dma_start(out=b2h[:], in_=B2Hd[:])
                b2l = st.tile([1, 512], F32R, name="b2l")
                nc.sync.dma_start(out=b2l[:], in_=B2Ld[:])
            if vth1_c is None:
                v1t = st.tile([128, 4 * NN], F32, name="v1t")
                nc.sync.dma_start(out=v1t[:], in_=V1d[:])
            if vthf_c is None:
                vft = st.tile([128, 4 * NN], F32, name="vft")
                nc.sync.dma_start(out=vft[:], in_=VFd[:])

            def plane(xt, rp, cp):
                off, nr, ncl = _PLANE[(rp, cp)]
                return xt[:, :, off:off + nr * ncl].rearrange(
                    "p b (y x) -> p b y x", y=nr)

            def dup_plane(xt, rp):
                off, nr = _DUP[rp]
                return xt[:, :, off:off + nr * 14].rearrange(
                    "p b (y x) -> p b y x", y=nr)

            def load_x(pair, t):
                xh, xl, xd = [], [], []
                ths, tls = [], []
                for cik in range(2):
                    th = xp.tile([128, NIMG * PLNH], BF16,
                                 name=f"xh_{pair}_{t}_{cik}", tag=f"xh{cik}")
                    tl = xp.tile([128, NIMG * PLN], BF16,
                                 name=f"xl_{pair}_{t}_{cik}", tag=f"xl{cik}")
                    ths.append(th)
                    tls.append(tl)
                    xh.append(th.rearrange("p (b f) -> p b f", b=NIMG))
                    xl.append(tl.rearrange("p (b f) -> p b f", b=NIMG))
                for cik in range(2):
                    for i in range(NIMG):
                        sl = slice(i * PLNH, (i + 1) * PLNH)
                        nc.sync.dma_start(out=ths[cik][:, sl],
                                          in_=XSHd[pair, t, cik][:, sl])
                for cik in range(2):
                    for i in range(NIMG):
                        sl = slice(i * PLN, (i + 1) * PLN)
                        nc.sync.dma_start(out=tls[cik][:, sl],
                                          in_=XSLd[pair, t, cik][:, sl])
                for cik in range(2):
                    td = xdp.tile([128, NIMG * 196], F32R,
                                  name=f"xd_{pair}_{t}_{cik}", tag=f"xd{cik}")
                    nc.sync.dma_start(out=td[:], in_=XDd[pair, t, cik])
                    xd.append(td.rearrange("p (b f) -> p b f", b=NIMG))
                return xh, xl, xd

            def load_xc(pair):
                xc, xcd = [], []
                for cik in range(2):
                    t = xp.tile([128, NIMG * PLNH], BF16,
                                name=f"xc_{pair}_{cik}", tag=f"xh{cik}")
                    nc.sync.dma_start(out=t[:], in_=XCd[pair, cik])
                    td = xdp.tile([128, NIMG * 196], F32R,
                                  name=f"xcd_{pair}_{cik}", tag=f"xd{cik}")
                    nc.sync.dma_start(out=td[:], in_=XCDd[pair, cik])
                    xc.append(t.rearrange("p (b f) -> p b f", b=NIMG))
                    xcd.append(td.rearrange("p (b f) -> p b f", b=NIMG))
                return xc, xcd

            def conv1_group(xh, xl, cok):
                """18 taps x 3 bf16 terms accumulating dense into one psum.
                Term-major order so the first 18 matmuls need only xh+w1h --
                shrinks the startup DMA critical set."""
                ps = pp.tile([128, NN], F32, name="ps1", tag="ps1")
                total = 54 + (2 if bias1_any else 0)
                n = 0
                for wsel, xsel in ((0, 0), (1, 0), (0, 1)):
                    for cik in range(2):
                        for ky in range(3):
                            rp, r0 = _tap_phase(ky)
                            for kx in range(3):
                                cp, c0 = _tap_phase(kx)
                                ti = ky * 3 + kx
                                ws = ((cok * 2 + cik) * 9 + ti) * 128
                                w_t = (w1h if wsel == 0 else w1l)[:, ws:ws + 128]
                                if xsel == 0 and kx == 2:
                                    r_t = dup_plane(xh[cik], rp)[
                                        :, :, r0:r0 + 14, 0:14]
                                else:
                                    xt = xh if xsel == 0 else xl
                                    r_t = plane(xt[cik], rp, cp)[
                                        :, :, r0:r0 + 14, c0:c0 + 14]
                                nc.tensor.matmul(ps[:], w_t, r_t,
                                                 start=(n == 0),
                                                 stop=(n == total - 1),
                                                 skip_group_check=True)
                                n += 1
                if bias1_any:
                    for b in (b1h, b1l):
                        nc.tensor.matmul(ps[:], b[0:1, cok * 128:(cok + 1) * 128],
                                         ones_b[:], start=False,
                                         stop=(n == total - 1),
                                         skip_group_check=True)
                        n += 1
                return ps

            def conv1_group_ann(xc, cok):
                """ANN conv1: single-term bf16 on resident w1 hi."""
                ps = pp.tile([128, NN], F32, name="ps1", tag="ps1")
                total = 18 + (2 if bias1_any else 0)
                n = 0
                for cik in range(2):
                    for ky in range(3):
                        rp, r0 = _tap_phase(ky)
                        for kx in range(3):
                            cp, c0 = _tap_phase(kx)
                            ti = ky * 3 + kx
                            ws = ((cok * 2 + cik) * 9 + ti) * 128
                            if kx == 2:
                                rhs = dup_plane(xc[cik], rp)[:, :, r0:r0 + 14,
                                                             0:14]
                            else:
                                rhs = plane(xc[cik], rp, cp)[:, :, r0:r0 + 14,
                                                             c0:c0 + 14]
                            nc.tensor.matmul(ps[:], w1h[:, ws:ws + 128], rhs,
                                             start=(n == 0),
                                             stop=(n == total - 1),
                                             skip_group_check=True)
                            n += 1
                if bias1_any:
                    for b in (b1h, b1l):
                        nc.tensor.matmul(ps[:], b[0:1, cok * 128:(cok + 1) * 128],
                                         ones_b[:], start=False,
                                         stop=(n == total - 1),
                                         skip_group_check=True)
                        n += 1
                return ps

            def conv2d_group(xd, avs, cok):
                """convd (2 f32r) + conv2 36 taps (f32r on padded o1/a views)."""
                ps = pp.tile([128, NN], F32, name="ps2", tag="ps2")
                total = 2 + 36 + (2 if bias2_any else 0)
                n = 0
                for cik in range(2):
                    nc.tensor.matmul(ps[:], wdr[:, (cok * 2 + cik) * 128:]
                                     [:, :128], xd[cik],
                                     start=(n == 0), stop=(n == total - 1),
                                     skip_group_check=True)
                    n += 1
                for cik in range(4):
                    for ky in range(3):
                        for kx in range(3):
                            ti = ky * 3 + kx
                            ws = ((cok * 4 + cik) * 9 + ti) * 128
                            rhs = o1_tap(avs[cik], ky, kx)
                            nc.tensor.matmul(ps[:], w2r[:, ws:ws + 128], rhs,
                                             start=False,
                                             stop=(n == total - 1),
                                             skip_group_check=True)
                            n += 1
                if bias2_any:
                    for b in (b2h, b2l):
                        nc.tensor.matmul(ps[:], b[0:1, cok * 128:(cok + 1) * 128],
                                         ones_r[:], start=False,
                                         stop=(n == total - 1),
                                         skip_group_check=True)
                        n += 1
                return ps

            def o1_views(par):
                return [o1p[k][par] for k in range(4)]

            def o1_tap(tile_, ky, kx):
                base = kx * 420 + ky * 14
                return tile_[:, base:base + 420].rearrange(
                    "p (b f) -> p b f", b=NIMG)[:, :, :196]

            def o1_dense(tile_, b):
                # contiguous 196-elem interior of the kx=1 (unshifted) copy
                return tile_[:, 420 + b * 210 + 14:][:, :196]

            def o1_make_shifts(tile_, b):
                """Replicate the written kx=1 copy into the kx=0/2 copies."""
                src = o1_dense(tile_, b).rearrange("p (y x) -> p y x", y=14)
                d0 = tile_[:, b * 210 + 14:][:, :196].rearrange(
                    "p (y x) -> p y x", y=14)[:, :, 1:14]
                nc.vector.tensor_copy(out=d0, in_=src[:, :, 0:13])
                d2 = tile_[:, 840 + b * 210 + 14:][:, :196].rearrange(
                    "p (y x) -> p y x", y=14)[:, :, 0:13]
                nc.vector.tensor_copy(out=d2, in_=src[:, :, 1:14])

            def scan1(ps_list, t):
                par = t % 2
                for k in range(4):
                    ps = ps_list[k]
                    tl = o1p[k][par]
                    if t == 0:
                        nc.vector.tensor_copy(out=mem1[k][:], in_=ps[:])
                    else:
                        nc.vector.tensor_add(out=mem1[k][:], in0=mem1[k][:],
                                             in1=ps[:])
                    if vth1_c is not None:
                        for b in range(NIMG):
                            m1s = mem1[k][:, b * PIX:(b + 1) * PIX]
                            nc.vector.tensor_scalar(out=o1_dense(tl, b),
                                                    in0=m1s, scalar1=vth1_c,
                                                    scalar2=None, op0=Alu.is_ge)
                        if t == 0:
                            nc.vector.tensor_scalar(out=mask1[k][:],
                                                    in0=mem1[k][:],
                                                    scalar1=vth1_c, scalar2=None,
                                                    op0=Alu.is_ge)
                        else:
                            nc.vector.scalar_tensor_tensor(
                                out=mask1[k][:], in0=mem1[k][:], scalar=vth1_c,
                                in1=mask1[k][:], op0=Alu.is_ge, op1=Alu.max)
                        if t < 3:
                            for b in range(NIMG):
                                m1s = mem1[k][:, b * PIX:(b + 1) * PIX]
                                nc.vector.scalar_tensor_tensor(
                                    out=m1s, in0=o1_dense(tl, b),
                                    scalar=-vth1_c, in1=m1s,
                                    op0=Alu.mult, op1=Alu.add)
                    else:
                        for b in range(NIMG):
                            m1s = mem1[k][:, b * PIX:(b + 1) * PIX]
                            vsv = v1t[:, k * NN + b * PIX:][:, :PIX]
                            nc.vector.tensor_tensor(out=o1_dense(tl, b),
                                                    in0=m1s, in1=vsv,
                                                    op=Alu.is_ge)
                        vs = v1t[:, k * NN:(k + 1) * NN]
                        if t == 0:
                            nc.vector.tensor_tensor(out=mask1[k][:],
                                                    in0=mem1[k][:], in1=vs,
                                                    op=Alu.is_ge)
                        else:
                            sc = iup.tile([128, NN], F32,
                                          name=f"s1_{t}_{k}", tag="iut")
                            nc.vector.tensor_tensor(out=sc[:],
                                                    in0=mem1[k][:], in1=vs,
                                                    op=Alu.is_ge)
                            nc.vector.tensor_max(out=mask1[k][:],
                                                 in0=mask1[k][:], in1=sc[:])
                        if t < 3:
                            for b in range(NIMG):
                                m1s = mem1[k][:, b * PIX:(b + 1) * PIX]
                                vsv = v1t[:, k * NN + b * PIX:][:, :PIX]
                                sc = iup.tile([128, NN], F32,
                                              name=f"s1b_{t}_{k}_{b}", tag="iut")
                                nc.vector.tensor_tensor(
                                    out=sc[:, :PIX], in0=o1_dense(tl, b),
                                    in1=vsv, op=Alu.mult)
                                nc.vector.tensor_sub(out=m1s, in0=m1s,
                                                     in1=sc[:, :PIX])
                    for b in range(NIMG):
                        o1_make_shifts(tl, b)

            def scanF(ps_list, t, pair):
                for k in range(4):
                    ps = ps_list[k]
                    if t == 0:
                        nc.vector.tensor_copy(out=memf[k][:], in_=ps[:])
                    else:
                        nc.vector.tensor_add(out=memf[k][:], in0=memf[k][:],
                                             in1=ps[:])
                    if t < 3:
                        o3_dst = iup.tile([128, NN], F32,
                                          name=f"o3s_{pair}_{t}_{k}",
                                          tag="iut")[:]
                    else:
                        o3t = ostr.tile([128, NN], F32, name=f"o3_{pair}_{k}",
                                        tag="ost")
                        o3_dst = o3t[:]
                    if vthf_c is not None:
                        nc.vector.tensor_scalar(out=o3_dst, in0=memf[k][:],
                                                scalar1=vthf_c, scalar2=None,
                                                op0=Alu.is_ge)
                        if t == 0:
                            nc.vector.tensor_scalar(out=mask3[k][:],
                                                    in0=memf[k][:],
                                                    scalar1=vthf_c, scalar2=None,
                                                    op0=Alu.is_ge)
                        else:
                            nc.vector.scalar_tensor_tensor(
                                out=mask3[k][:], in0=memf[k][:], scalar=vthf_c,
                                in1=mask3[k][:], op0=Alu.is_ge, op1=Alu.max)
                        if t < 3:
                            nc.vector.scalar_tensor_tensor(
                                out=memf[k][:], in0=o3_dst, scalar=-vthf_c,
                                in1=memf[k][:], op0=Alu.mult, op1=Alu.add)
                    else:
                        vs = vft[:, k * NN:(k + 1) * NN]
                        nc.vector.tensor_tensor(out=o3_dst, in0=memf[k][:],
                                                in1=vs, op=Alu.is_ge)
                        if t == 0:
                            nc.vector.tensor_copy(out=mask3[k][:], in_=o3_dst)
                        else:
                            nc.vector.tensor_max(out=mask3[k][:],
                                                 in0=mask3[k][:], in1=o3_dst)
                        if t < 3:
                            nc.vector.tensor_tensor(out=o3_dst, in0=o3_dst,
                                                    in1=vs, op=Alu.mult)
                            nc.vector.tensor_sub(out=memf[k][:], in0=memf[k][:],
                                                 in1=o3_dst)
                    if t == 3:
                        nc.sync.dma_start(
                            out=O3d[pair][:, k * NN:(k + 1) * NN], in_=o3_dst)
                        iut = iup.tile([128, NN], F32, name=f"iu_{pair}_{k}",
                                       tag="iut")
                        nc.scalar.copy(out=iut[:], in_=ps[:])
                        nc.sync.dma_start(out=IUd[pair, k], in_=iut[:])

            # Startup DMA order: pair-0 x(app 0) first, then w1 cok-chunks
            # in fine sub-chunks (a single DMA stream runs ~57 GB/s, so the
            # largest single transfer gates the first matmul), x(app 1),
            # o1p zero rings (needed by scan1(0) ~+55us), wdr, then w2r
            # cok-chunks (first needed ~+100us).
            # p-state warmup: junk matmuls on zeroed tiles keep the PE's HAM
            # clock gate open through the startup DMA window (~17.5us); the
            # first real conv1 then issues at full 2.4GHz instead of ramping
            # (~3us saved). Sized to end at data-arrival: 48 x 392-col
            # (covers the cold-start ramp + bulk) then 24 x 64-col for a
            # fine-grained tail (overrun costs at most ~56ns each).
            wtmp = st.tile([128, 128], BF16, name="wtmp")
            xtmp = st.tile([128, NN], BF16, name="xtmp")
            nc.vector.memset(wtmp[:], 0.0)
            nc.vector.memset(xtmp[:], 0.0)
            pwarm = pp.tile([128, NN], F32, name="warm", tag="ps1")
            for i in range(44):
                nc.tensor.matmul(pwarm[:], wtmp[:], xtmp[:], start=(i == 0),
                                 stop=False, skip_group_check=True)
            for i in range(16):
                nc.tensor.matmul(pwarm[:, :64], wtmp[:], xtmp[:, :64],
                                 start=False, stop=(i == 15),
                                 skip_group_check=True)
            wrd = iup.tile([128, NN], F32, name="warmrd", tag="iut")
            nc.scalar.copy(out=wrd[:], in_=pwarm[:])

            xv00 = load_x(0, 0)
            CW1 = 2 * 9 * 128
            for q in range(4):
                nsub = 4 if q == 0 else 1
                for h in range(nsub):
                    sl = slice(q * CW1 + h * CW1 // nsub,
                               q * CW1 + (h + 1) * CW1 // nsub)
                    nc.sync.dma_start(out=w1h[:, sl], in_=W1Hd[:, sl])
                for h in range(nsub):
                    sl = slice(q * CW1 + h * CW1 // nsub,
                               q * CW1 + (h + 1) * CW1 // nsub)
                    nc.sync.dma_start(out=w1l[:, sl], in_=W1Ld[:, sl])
            for k in range(4):
                for par in range(2):
                    nc.sync.dma_start(out=o1p[k][par][:], in_=ZPADd[:])
            xv01 = load_x(0, 1)
            nc.sync.dma_start(out=wdr[:], in_=WDRd[:])
            CW2 = 4 * 9 * 128
            for q in range(4):
                sl = slice(q * CW2, (q + 1) * CW2)
                nc.sync.dma_start(out=w2r[:, sl], in_=W2Rd[:, sl])

            for pair in range(NPAIR):
                xv = {}
                if pair == 0:
                    xv[0], xv[1] = xv00, xv01
                else:
                    xv[0] = load_x(pair, 0)
                    xv[1] = load_x(pair, 1)

                ps1 = {0: [conv1_group(xv[0][0], xv[0][1], k) for k in range(4)]}
                scan1(ps1[0], 0)

                ps1[1] = [conv1_group(xv[1][0], xv[1][1], k) for k in range(4)]
                ps2 = {0: [conv2d_group(xv[0][2], o1_views(0), k)
                           for k in range(4)]}
                xv[2] = load_x(pair, 2)
                scan1(ps1[1], 1)
                scanF(ps2[0], 0, pair)

                ps1[2] = [conv1_group(xv[2][0], xv[2][1], k) for k in range(4)]
                ps2[1] = [conv2d_group(xv[1][2], o1_views(1), k)
                          for k in range(4)]
                xv[3] = load_x(pair, 3)
                scan1(ps1[2], 2)
                scanF(ps2[1], 1, pair)

                ps1[3] = [conv1_group(xv[3][0], xv[3][1], k) for k in range(4)]
                ps2[2] = [conv2d_group(xv[2][2], o1_views(0), k)
                          for k in range(4)]
                xc, xcd = load_xc(pair)
                scan1(ps1[3], 3)
                scanF(ps2[2], 2, pair)

                ps2[3] = [conv2d_group(xv[3][2], o1_views(1), k)
                          for k in range(4)]
                scanF(ps2[3], 3, pair)

                # ANN branch: a = relu(conv1(inp_c)) * mask1 into the parity-0
                # padded tiles (free after conv2d of t=2), then
                # out_c = relu(conv2(a) + convd(inp_c)) * mask3.
                ps_c1 = [conv1_group_ann(xc, k) for k in range(4)]
                for k in range(4):
                    tl = o1p[k][0]
                    sc = iup.tile([128, NN], F32, name=f"ar_{pair}_{k}",
                                  tag="iut")
                    nc.scalar.activation(sc[:], ps_c1[k][:], Act.Relu)
                    for b in range(NIMG):
                        nc.vector.tensor_tensor(
                            out=o1_dense(tl, b),
                            in0=sc[:, b * PIX:(b + 1) * PIX],
                            in1=mask1[k][:, b * PIX:(b + 1) * PIX],
                            op=Alu.mult)
                        o1_make_shifts(tl, b)
                ps_c2 = [conv2d_group(xcd, o1_views(0), k)
                         for k in range(4)]
                for k in range(4):
                    oct_ = ostr.tile([128, NN], F32, name=f"oc_{pair}_{k}",
                                     tag="ost")
                    nc.scalar.activation(oct_[:], ps_c2[k][:], Act.Relu)
                    nc.vector.tensor_tensor(out=oct_[:], in0=oct_[:],
                                            in1=mask3[k][:], op=Alu.mult)
                    for b in range(NIMG):
                        sl = slice(b * PIX, (b + 1) * PIX)
                        nc.sync.dma_start(
                            out=OCd[pair][:, k * NN + b * PIX:][:, :PIX],
                            in_=oct_[:, sl])

    nc.finalize()
    return nc


def _pack_weights(w):
    # cok-major: [128part=cin_low, (cok, cik, tap, 128 cout_low)] so per-cok
    # DMA chunks unlock conv groups incrementally at startup.
    Co, Ci, kh, kw = w.shape
    nchunk = Ci // 128
    a = w.reshape(4, 128, nchunk, 128, kh * kw)  # [cok, co_low, cik, ci_low, tap]
    return np.ascontiguousarray(
        a.transpose(3, 0, 2, 4, 1).reshape(128, 4 * nchunk * kh * kw * 128))


def _vth_const(v):
    v = np.asarray(v, np.float32)
    return float(v.flat[0]) if np.all(v == v.flat[0]) else None


def _vth_rep(v):
    # [512,14,14] -> [128, (chunk, img, pix)] replicated over the image pair
    a = np.asarray(v, np.float32).reshape(4, 128, PIX)
    a = np.broadcast_to(a[:, None, :, :], (4, NIMG, 128, PIX))
    return np.ascontiguousarray(a.transpose(2, 0, 1, 3).reshape(128, 4 * NN))


def _bf_split(a):
    h = a.astype(ml_dtypes.bfloat16)
    l = (a - h.astype(np.float32)).astype(ml_dtypes.bfloat16)
    return np.ascontiguousarray(h), np.ascontiguousarray(l)


def _r_split(a):
    """f32r (11-mantissa-bit) hi/lo split, both stored as fp32 bits."""
    i = a.astype(np.float32).view(np.uint32).astype(np.uint64)
    i = (i + ((i >> 12) & 1) + 0x7FF) & np.uint64(0xFFFFF000)
    h = i.astype(np.uint32).view(np.float32)
    return np.ascontiguousarray(h), np.ascontiguousarray(a - h)


def _planes(x, dup=False):
    """[..., 28, 28] -> [..., 842 or 1248] zero-padded stride-2 parity planes
    (15x15, 15x14, 14x15, 14x14 + tail pad). With dup=True, appends 14-wide
    copies of the even-column planes shifted one column (for kx=2 taps:
    run starts stay 4-byte aligned and 14-row reads are contiguous)."""
    sh = x.shape[:-2]
    xp = np.zeros(sh + (30, 30), np.float32)
    xp[..., 1:29, 1:29] = x
    n = 1248 if dup else 842
    out = np.zeros(sh + (n,), np.float32)
    out[..., 0:225] = xp[..., 0:30:2, 0:30:2].reshape(sh + (225,))
    out[..., 225:435] = xp[..., 0:30:2, 1:29:2].reshape(sh + (210,))
    out[..., 435:645] = xp[..., 1:29:2, 0:30:2].reshape(sh + (210,))
    out[..., 645:841] = xp[..., 1:29:2, 1:29:2].reshape(sh + (196,))
    if dup:
        out[..., 842:1052] = xp[..., 0:30:2, 2:30:2].reshape(sh + (210,))
        out[..., 1052:1248] = xp[..., 1:29:2, 2:30:2].reshape(sh + (196,))
    return out


def kernel(inp_s, inp_u, inp_c, conv1_w, conv2_w, ds_w,
           bn1_gamma, bn1_beta, bn1_mean, bn1_var,
           bn2_gamma, bn2_beta, bn2_mean, bn2_var,
           dsbn_gamma, dsbn_beta, dsbn_mean, dsbn_var,
           vth1, vth2, vth_ds, vth_if):
    global LAST_RESULT
    f32 = lambda x: np.asarray(x, np.float32)
    inp_s, inp_c = f32(inp_s), f32(inp_c)

    def fold(w, gamma, beta, mean, var):
        s = f32(gamma) / np.sqrt(f32(var) + np.float32(EPS))
        return f32(w) * s[:, None, None, None], f32(beta) - f32(mean) * s

    w1, b1 = fold(conv1_w, bn1_gamma, bn1_beta, bn1_mean, bn1_var)
    w2, b2 = fold(conv2_w, bn2_gamma, bn2_beta, bn2_mean, bn2_var)
    wd, bd = fold(ds_w, dsbn_gamma, dsbn_beta, dsbn_mean, dsbn_var)
    b2d = b2 + bd

    vth1_c = _vth_const(vth1)
    vthf_c = _vth_const(vth_if)
    bias1_any = bool(np.any(b1 != 0))
    bias2_any = bool(np.any(b2d != 0))

    cfg = (bias1_any, bias2_any, vth1_c, vthf_c)
    if cfg not in _CACHE:
        _CACHE[cfg] = _build(cfg)
    nc = _CACHE[cfg]

    W1H, W1L = _bf_split(_pack_weights(w1))
    W2R = _pack_weights(w2)
    WDR = _pack_weights(wd)

    T, B = inp_s.shape[:2]
    # planes + bf16 hi/lo split; hi carries the kx=2 dup planes (same bf16
    # rounding, so dup values match the main planes bit-for-bit)
    pl_s = _planes(inp_s, dup=True)
    hi_s = pl_s.astype(ml_dtypes.bfloat16)
    lo_s = (pl_s[..., :842] - hi_s[..., :842].astype(np.float32)) \
        .astype(ml_dtypes.bfloat16)
    xd_s = np.ascontiguousarray(inp_s[..., 0::2, 0::2].reshape(T, B, 256, 196))
    pl_c = _planes(inp_c, dup=True).astype(ml_dtypes.bfloat16)
    xcd_c = np.ascontiguousarray(inp_c[..., 0::2, 0::2].reshape(B, 256, 196))

    def pack_xs(a, f):
        # [T, 4img, 256, f] -> [NPAIR, T, 2cik, 128, NIMG*f]
        r = a.reshape(T, NPAIR, NIMG, 2, 128, f)
        return np.ascontiguousarray(
            r.transpose(1, 0, 3, 4, 2, 5).reshape(NPAIR, T, 2, 128, NIMG * f))

    def pack_xc(a, f):
        # [4img, 256, f] -> [NPAIR, 2cik, 128, NIMG*f]
        r = a.reshape(NPAIR, NIMG, 2, 128, f)
        return np.ascontiguousarray(
            r.transpose(0, 2, 3, 1, 4).reshape(NPAIR, 2, 128, NIMG * f))

    in_maps = []
    for core in range(NCORES):
        b0 = core * BPC
        m = {
            "W1H": W1H, "W1L": W1L, "W2R": W2R, "WDR": WDR,
            "XSH": pack_xs(hi_s[:, b0:b0 + BPC], 1248),
            "XSL": pack_xs(lo_s[:, b0:b0 + BPC], 842),
            "XD": pack_xs(xd_s[:, b0:b0 + BPC], 196),
            "XC": pack_xc(pl_c[b0:b0 + BPC], 1248),
            "ZPAD": np.zeros((128, 1288), np.float32),
            "XCD": pack_xc(xcd_c[b0:b0 + BPC], 196),
        }
        if bias1_any:
            bh, bl = _bf_split(b1.reshape(1, 512))
            m["B1H"], m["B1L"] = bh, bl
        if bias2_any:
            bh, bl = _r_split(b2d.reshape(1, 512))
            m["B2H"], m["B2L"] = bh, bl
            m["ONER"] = np.ones((1, NN), np.float32)
        if vth1_c is None:
            m["VTH1R"] = _vth_rep(vth1)
        if vthf_c is None:
            m["VTHFR"] = _vth_rep(vth_if)
        in_maps.append(m)

    from concourse.bass_utils import run_bass_kernel_spmd
    if TRACE:
        try:
            from trn_agent_boot.trn_boot import _ntff_profile_via_ctypes
            from antenv.axon_hooks import set_axon_ntff_profile_hook
            set_axon_ntff_profile_hook(
                _ntff_profile_via_ctypes('/opt/axon/libaxon_pjrt.so'))
        except Exception:
            pass
    res = run_bass_kernel_spmd(nc, in_maps, core_ids=list(range(NCORES)),
                               trace=TRACE)
    LAST_RESULT = res

    o3 = np.empty((B, 512, 14, 14), np.float32)
    iu = np.empty((B, 512, 14, 14), np.float32)
    oc = np.empty((B, 512, 14, 14), np.float32)
    for core in range(NCORES):
        b0 = core * BPC
        for name, dst in (("O3", o3), ("OC", oc)):
            arr = res.results[core][name].reshape(NPAIR, 128, 4, NIMG, PIX)
            arr = arr.transpose(0, 3, 2, 1, 4).reshape(BPC, 512, 14, 14)
            dst[b0:b0 + BPC] = arr
        arr = res.results[core]["IU"].reshape(NPAIR, 4, 128, NIMG, PIX)
        arr = arr.transpose(0, 3, 1, 2, 4).reshape(BPC, 512, 14, 14)
        iu[b0:b0 + BPC] = arr
    return o3, iu, oc


# revision 35
# speedup vs baseline: 1.0022x; 1.0022x over previous
"""Trainium2 Bass kernel for nn_BasicBlock_88665304858673 (spiking BasicBlock).

Computation (dead code removed -- mem2/o2/m2, memd/od and inp_u never reach
the outputs):

  per time step t (T=4):
    I1_t   = conv1(x_t)            3x3 stride2 pad1, 256->512, BN-folded
    mem1  += I1_t ; o1_t = (mem1 >= vth1) ; mem1 -= o1_t*vth1 ; mask1 |= o1_t
    out_s_t = conv2(o1_t) + convd(x_t)     (3x3 s1 p1 and 1x1 s2)
    memf  += out_s_t ; o3_t = (memf >= vth_if) ; memf -= o3_t*vth_if ; mask3 |= o3_t
  outputs: o3_3, out_s_3, and the ANN branch
    a     = relu(conv1(inp_c)) * mask1
    out_c = relu(conv2(a) + convd(inp_c)) * mask3

Sharding: data-parallel over batch B=32 -> 8 cores x 4 images (2 pairs of 2;
matmul moving dim N = 2*196 = 392).

Matmul dtype plan (fp32 runs at 4 cycles/col on the PE; bf16 and f32r --
fp32 bits rounded to 11 mantissa bits by the PE -- run at 1 cycle/col):
  conv1  : bf16 3-term split  xh*wh + xh*wl + xl*wh   (~2^-17 accurate; conv1
           feeds the mem1 threshold whose spike flips amplify through conv2,
           so single-rounded terms are NOT enough -- measured in simulation)
  conv2  : f32r single term. Input o1 is binary {0,1} => exact in any dtype;
           only the weight sees the 2^-12 rounding, and errors here only
           shift memf slightly (simulated combined rel err ~7e-3 vs 2e-2 gate)
  convd  : f32r single term (same argument, small magnitude)
  ANN    : conv1 bf16 single (reuses resident w1-hi), conv2/convd f32r
           (reuse resident f32r weights); no thresholds downstream.

All matmuls are dense [128, 392]: inputs are host-side phase-decomposed into
the four stride-2 parity planes of the zero-padded 30x30 image (sizes 15x15,
15x14, 14x15, 14x14 concat -> 842), so each conv1 tap reads a stride-1 14x14
window and writes the full psum tile. Zero padding makes every tap valid
everywhere and keeps innermost runs even (an f32r moving-operand requirement).

o1 spikes are stored as 3 kx-pre-shifted zero-padded copies so every conv2
tap reads two fully CONTIGUOUS 196-element runs -- strided f32r moving
operands cost ~2 PE cycles per run transition (measured 202ns vs 180ns per
392-col matmul).

Weights stay resident in SBUF for the whole kernel (loaded once, cok-major
so per-cok DMA chunks unlock the first conv groups early): w1 hi/lo bf16,
w2/wd f32r. Measured: 1266927ns (fp32 baseline) -> 627352ns, ~97% tensor
engine occupancy, rel err 1.18e-2 (gate 2e-2, bit-stable across runs).
"""

import numpy as np
import ml_dtypes

EPS = 1e-5
NCORES = 8
BPC = 4          # images per core
NPAIR = 2        # image pairs per core
NIMG = 2         # images per pair
PIX = 196
NN = NIMG * PIX  # moving dim: 392

_CACHE = {}
TRACE = False
LAST_RESULT = None

# parity plane (rp, cp) -> (offset, nrows, ncols) within the per-image layout.
# Row widths are padded even (16/14/16/14) and the image stride (872) is even
# so every bf16 row start lands 4-byte aligned.
_PLANE = {
    (0, 0): (0, 15, 15),
    (0, 1): (225, 15, 14),
    (1, 0): (435, 14, 15),
    (1, 1): (645, 14, 14),
}
PLN = 842      # xl layout: 4 parity planes + tail pad
PLNH = 1248    # xh/xc layout: planes + 14-wide kx=2 duplicates at 842/1052
# rp -> (dup offset, nrows); dup[r, c] = plane(rp,0)[r, c+1], rows 14-wide so
# every run start is even and a 14-row read is fully contiguous
_DUP = {0: (842, 15), 1: (1052, 14)}


def _tap_phase(k):
    """3x3 tap index along one axis -> (parity, start offset in plane)."""
    return (1, 0) if k == 1 else (0, 1 if k == 2 else 0)


def _build(cfg):
    """cfg = (bias1_any, bias2_any, vth1_scalar_or_None, vthf_scalar_or_None)"""
    import concourse.bacc as bacc
    import concourse.mybir as mybir
    import concourse.tile as tile

    F32 = mybir.dt.float32
    F32R = mybir.dt.float32r
    BF16 = mybir.dt.bfloat16
    Alu = mybir.AluOpType
    Act = mybir.ActivationFunctionType
    bias1_any, bias2_any, vth1_c, vthf_c = cfg

    nc = bacc.Bacc(None, target_bir_lowering=False)

    W1Hd = nc.dram_tensor("W1H", [128, 2 * 9 * 512], BF16, kind="ExternalInput")
    W1Ld = nc.dram_tensor("W1L", [128, 2 * 9 * 512], BF16, kind="ExternalInput")
    W2Rd = nc.dram_tensor("W2R", [128, 4 * 9 * 512], F32R, kind="ExternalInput")
    WDRd = nc.dram_tensor("WDR", [128, 2 * 512], F32R, kind="ExternalInput")
    XSHd = nc.dram_tensor("XSH", [NPAIR, 4, 2, 128, NIMG * PLNH], BF16,
                          kind="ExternalInput")
    XSLd = nc.dram_tensor("XSL", [NPAIR, 4, 2, 128, NIMG * PLN], BF16,
                          kind="ExternalInput")
    XDd = nc.dram_tensor("XD", [NPAIR, 4, 2, 128, NIMG * 196], F32R,
                         kind="ExternalInput")
    XCd = nc.dram_tensor("XC", [NPAIR, 2, 128, NIMG * PLNH], BF16,
                         kind="ExternalInput")
    XCDd = nc.dram_tensor("XCD", [NPAIR, 2, 128, NIMG * 196], F32R,
                          kind="ExternalInput")
    ZPADd = nc.dram_tensor("ZPAD", [128, 1288], F32R, kind="ExternalInput")
    if bias1_any:
        B1Hd = nc.dram_tensor("B1H", [1, 512], BF16, kind="ExternalInput")
        B1Ld = nc.dram_tensor("B1L", [1, 512], BF16, kind="ExternalInput")
    if bias2_any:
        B2Hd = nc.dram_tensor("B2H", [1, 512], F32R, kind="ExternalInput")
        B2Ld = nc.dram_tensor("B2L", [1, 512], F32R, kind="ExternalInput")
        ONERd = nc.dram_tensor("ONER", [1, NN], F32R, kind="ExternalInput")
    if vth1_c is None:
        V1d = nc.dram_tensor("VTH1R", [128, 4 * NN], F32, kind="ExternalInput")
    if vthf_c is None:
        VFd = nc.dram_tensor("VTHFR", [128, 4 * NN], F32, kind="ExternalInput")
    O3d = nc.dram_tensor("O3", [NPAIR, 128, 4 * NN], F32, kind="ExternalOutput")
    IUd = nc.dram_tensor("IU", [NPAIR, 4, 128, NN], F32, kind="ExternalOutput")
    OCd = nc.dram_tensor("OC", [NPAIR, 128, 4 * NN], F32, kind="ExternalOutput")

    with tile.TileContext(nc) as tc:
        with tc.tile_pool(name="wpool", bufs=1) as wp, \
             tc.tile_pool(name="xpool", bufs=2) as xp, \
             tc.tile_pool(name="spool", bufs=1) as st, \
             tc.tile_pool(name="ostr", bufs=1) as ostr, \
             tc.tile_pool(name="xdpool", bufs=1) as xdp, \
             tc.tile_pool(name="iupool", bufs=1) as iup, \
             tc.tile_pool(name="pspool", bufs=4, space="PSUM") as pp:

            w1h = wp.tile([128, 2 * 9 * 512], BF16, name="w1h")
            w1l = wp.tile([128, 2 * 9 * 512], BF16, name="w1l")
            w2r = wp.tile([128, 4 * 9 * 512], F32R, name="w2r")
            wdr = wp.tile([128, 2 * 512], F32R, name="wdr")

            mem1 = [st.tile([128, NN], F32, name=f"mem1_{k}") for k in range(4)]
            memf = [st.tile([128, NN], F32, name=f"memf_{k}") for k in range(4)]
            U8 = mybir.dt.uint8
            mask1 = [st.tile([128, NN], U8, name=f"mask1_{k}") for k in range(4)]
            mask3 = [st.tile([128, NN], U8, name=f"mask3_{k}") for k in range(4)]
            # o1 spikes stored as 3 kx-pre-shifted copies (one per conv2
            # column tap), each [2 img, 15 rows, 14 cols] zero-padded, plus a
            # 28-elem zero tail: copy kx at [kx*420 + b*210 + y*14 + x] holds
            # o1[b, y-1, x-1+kx].  A tap (ky,kx) then reads TWO CONTIGUOUS
            # 196-elem runs (one per image) -- f32r matmuls pay ~2 cycles per
            # rhs run transition, so 28 short runs cost ~24ns/matmul extra.
            # Vertical overruns (row 15) land on the next block's zero pad row.
            # Double-buffered by timestep parity; parity-0 slots are reused
            # for the ANN gated-relu input.
            o1p = [[st.tile([128, 1288], F32R, name=f"o1p_{k}_{par}")
                    for par in range(2)] for k in range(4)]


            if bias1_any:
                ones_b = st.tile([1, NN], BF16, name="ones_b")
                nc.vector.memset(ones_b[:], 1.0)
                b1h = st.tile([1, 512], BF16, name="b1h")
                nc.sync.dma_start(out=b1h[:], in_=B1Hd[:])
                b1l = st.tile([1, 512], BF16, name="b1l")
                nc.sync.dma_start(out=b1l[:], in_=B1Ld[:])
            if bias2_any:
                ones_r = st.tile([1, NN], F32R, name="ones_r")
                nc.sync.dma_start(out=ones_r[:], in_=ONERd[:])
                b2h = st.tile([1, 512], F32R, name="b2h")
                nc.sync.dma_start(out=b2h[:], in_=B2Hd[:])
                b2l = st.tile([1, 512], F32R, name="b2l")
                nc.sync.dma_start(out=b2l[:], in_=B2Ld[:])
            if vth1_c is None:
                v1t = st.tile([128, 4 * NN], F32, name="v1t")
                nc.sync.dma_start(out=v1t[:], in_=V1d[:])
            if vthf_c is None:
                vft = st.tile([128, 4 * NN], F32, name="vft")
                nc.sync.dma_start(out=vft[:], in_=VFd[:])

            def plane(xt, rp, cp):
                off, nr, ncl = _PLANE[(rp, cp)]
                return xt[:, :, off:off + nr * ncl].rearrange(
                    "p b (y x) -> p b y x", y=nr)

            def dup_plane(xt, rp):
                off, nr = _DUP[rp]
                return xt[:, :, off:off + nr * 14].rearrange(
                    "p b (y x) -> p b y x", y=nr)

            def load_x(pair, t):
                xh, xl, xd = [], [], []
                ths, tls = [], []
                for cik in range(2):
                    th = xp.tile([128, NIMG * PLNH], BF16,
                                 name=f"xh_{pair}_{t}_{cik}", tag=f"xh{cik}")
                    tl = xp.tile([128, NIMG * PLN], BF16,
                                 name=f"xl_{pair}_{t}_{cik}", tag=f"xl{cik}")
                    ths.append(th)
                    tls.append(tl)
                    xh.append(th.rearrange("p (b f) -> p b f", b=NIMG))
                    xl.append(tl.rearrange("p (b f) -> p b f", b=NIMG))
                for cik in range(2):
                    for i in range(NIMG):
                        sl = slice(i * PLNH, (i + 1) * PLNH)
                        nc.sync.dma_start(out=ths[cik][:, sl],
                                          in_=XSHd[pair, t, cik][:, sl])
                for cik in range(2):
                    for i in range(NIMG):
                        sl = slice(i * PLN, (i + 1) * PLN)
                        nc.sync.dma_start(out=tls[cik][:, sl],
                                          in_=XSLd[pair, t, cik][:, sl])
                for cik in range(2):
                    td = xdp.tile([128, NIMG * 196], F32R,
                                  name=f"xd_{pair}_{t}_{cik}", tag=f"xd{cik}")
                    nc.sync.dma_start(out=td[:], in_=XDd[pair, t, cik])
                    xd.append(td.rearrange("p (b f) -> p b f", b=NIMG))
                return xh, xl, xd

            def load_xc(pair):
                xc, xcd = [], []
                for cik in range(2):
                    t = xp.tile([128, NIMG * PLNH], BF16,
                                name=f"xc_{pair}_{cik}", tag=f"xh{cik}")
                    nc.sync.dma_start(out=t[:], in_=XCd[pair, cik])
                    td = xdp.tile([128, NIMG * 196], F32R,
                                  name=f"xcd_{pair}_{cik}", tag=f"xd{cik}")
                    nc.sync.dma_start(out=td[:], in_=XCDd[pair, cik])
                    xc.append(t.rearrange("p (b f) -> p b f", b=NIMG))
                    xcd.append(td.rearrange("p (b f) -> p b f", b=NIMG))
                return xc, xcd

            def conv1_group(xh, xl, cok):
                """18 taps x 3 bf16 terms accumulating dense into one psum.
                Term-major order so the first 18 matmuls need only xh+w1h --
                shrinks the startup DMA critical set."""
                ps = pp.tile([128, NN], F32, name="ps1", tag="ps1")
                total = 54 + (2 if bias1_any else 0)
                n = 0
                for wsel, xsel in ((0, 0), (1, 0), (0, 1)):
                    for cik in range(2):
                        for ky in range(3):
                            rp, r0 = _tap_phase(ky)
                            for kx in range(3):
                                cp, c0 = _tap_phase(kx)
                                ti = ky * 3 + kx
                                ws = ((cok * 2 + cik) * 9 + ti) * 128
                                w_t = (w1h if wsel == 0 else w1l)[:, ws:ws + 128]
                                if xsel == 0 and kx == 2:
                                    r_t = dup_plane(xh[cik], rp)[
                                        :, :, r0:r0 + 14, 0:14]
                                else:
                                    xt = xh if xsel == 0 else xl
                                    r_t = plane(xt[cik], rp, cp)[
                                        :, :, r0:r0 + 14, c0:c0 + 14]
                                nc.tensor.matmul(ps[:], w_t, r_t,
                                                 start=(n == 0),
                                                 stop=(n == total - 1),
                                                 skip_group_check=True)
                                n += 1
                if bias1_any:
                    for b in (b1h, b1l):
                        nc.tensor.matmul(ps[:], b[0:1, cok * 128:(cok + 1) * 128],
                                         ones_b[:], start=False,
                                         stop=(n == total - 1),
                                         skip_group_check=True)
                        n += 1
                return ps

            def conv1_group_ann(xc, cok):
                """ANN conv1: single-term bf16 on resident w1 hi."""
                ps = pp.tile([128, NN], F32, name="ps1", tag="ps1")
                total = 18 + (2 if bias1_any else 0)
                n = 0
                for cik in range(2):
                    for ky in range(3):
                        rp, r0 = _tap_phase(ky)
                        for kx in range(3):
                            cp, c0 = _tap_phase(kx)
                            ti = ky * 3 + kx
                            ws = ((cok * 2 + cik) * 9 + ti) * 128
                            if kx == 2:
                                rhs = dup_plane(xc[cik], rp)[:, :, r0:r0 + 14,
                                                             0:14]
                            else:
                                rhs = plane(xc[cik], rp, cp)[:, :, r0:r0 + 14,
                                                             c0:c0 + 14]
                            nc.tensor.matmul(ps[:], w1h[:, ws:ws + 128], rhs,
                                             start=(n == 0),
                                             stop=(n == total - 1),
                                             skip_group_check=True)
                            n += 1
                if bias1_any:
                    for b in (b1h, b1l):
                        nc.tensor.matmul(ps[:], b[0:1, cok * 128:(cok + 1) * 128],
                                         ones_b[:], start=False,
                                         stop=(n == total - 1),
                                         skip_group_check=True)
                        n += 1
                return ps

            def conv2d_group(xd, avs, cok):
                """convd (2 f32r) + conv2 36 taps (f32r on padded o1/a views)."""
                ps = pp.tile([128, NN], F32, name="ps2", tag="ps2")
                total = 2 + 36 + (2 if bias2_any else 0)
                n = 0
                for cik in range(2):
                    nc.tensor.matmul(ps[:], wdr[:, (cok * 2 + cik) * 128:]
                                     [:, :128], xd[cik],
                                     start=(n == 0), stop=(n == total - 1),
                                     skip_group_check=True)
                    n += 1
                for cik in range(4):
                    for ky in range(3):
                        for kx in range(3):
                            ti = ky * 3 + kx
                            ws = ((cok * 4 + cik) * 9 + ti) * 128
                            rhs = o1_tap(avs[cik], ky, kx)
                            nc.tensor.matmul(ps[:], w2r[:, ws:ws + 128], rhs,
                                             start=False,
                                             stop=(n == total - 1),
                                             skip_group_check=True)
                            n += 1
                if bias2_any:
                    for b in (b2h, b2l):
                        nc.tensor.matmul(ps[:], b[0:1, cok * 128:(cok + 1) * 128],
                                         ones_r[:], start=False,
                                         stop=(n == total - 1),
                                         skip_group_check=True)
                        n += 1
                return ps

            def o1_views(par):
                return [o1p[k][par] for k in range(4)]

            def o1_tap(tile_, ky, kx):
                base = kx * 420 + ky * 14
                return tile_[:, base:base + 420].rearrange(
                    "p (b f) -> p b f", b=NIMG)[:, :, :196]

            def o1_dense(tile_, b):
                # contiguous 196-elem interior of the kx=1 (unshifted) copy
                return tile_[:, 420 + b * 210 + 14:][:, :196]

            def o1_make_shifts(tile_, b):
                """Replicate the written kx=1 copy into the kx=0/2 copies."""
                src = o1_dense(tile_, b).rearrange("p (y x) -> p y x", y=14)
                d0 = tile_[:, b * 210 + 14:][:, :196].rearrange(
                    "p (y x) -> p y x", y=14)[:, :, 1:14]
                nc.vector.tensor_copy(out=d0, in_=src[:, :, 0:13])
                d2 = tile_[:, 840 + b * 210 + 14:][:, :196].rearrange(
                    "p (y x) -> p y x", y=14)[:, :, 0:13]
                nc.vector.tensor_copy(out=d2, in_=src[:, :, 1:14])

            def scan1(ps_list, t):
                par = t % 2
                for k in range(4):
                    ps = ps_list[k]
                    tl = o1p[k][par]
                    if t == 0:
                        nc.vector.tensor_copy(out=mem1[k][:], in_=ps[:])
                    else:
                        nc.vector.tensor_add(out=mem1[k][:], in0=mem1[k][:],
                                             in1=ps[:])
                    if vth1_c is not None:
                        for b in range(NIMG):
                            m1s = mem1[k][:, b * PIX:(b + 1) * PIX]
                            nc.vector.tensor_scalar(out=o1_dense(tl, b),
                                                    in0=m1s, scalar1=vth1_c,
                                                    scalar2=None, op0=Alu.is_ge)
                        if t == 0:
                            nc.vector.tensor_scalar(out=mask1[k][:],
                                                    in0=mem1[k][:],
                                                    scalar1=vth1_c, scalar2=None,
                                                    op0=Alu.is_ge)
                        else:
                            nc.vector.scalar_tensor_tensor(
                                out=mask1[k][:], in0=mem1[k][:], scalar=vth1_c,
                                in1=mask1[k][:], op0=Alu.is_ge, op1=Alu.max)
                        if t < 3:
                            for b in range(NIMG):
                                m1s = mem1[k][:, b * PIX:(b + 1) * PIX]
                                nc.vector.scalar_tensor_tensor(
                                    out=m1s, in0=o1_dense(tl, b),
                                    scalar=-vth1_c, in1=m1s,
                                    op0=Alu.mult, op1=Alu.add)
                    else:
                        for b in range(NIMG):
                            m1s = mem1[k][:, b * PIX:(b + 1) * PIX]
                            vsv = v1t[:, k * NN + b * PIX:][:, :PIX]
                            nc.vector.tensor_tensor(out=o1_dense(tl, b),
                                                    in0=m1s, in1=vsv,
                                                    op=Alu.is_ge)
                        vs = v1t[:, k * NN:(k + 1) * NN]
                        if t == 0:
                            nc.vector.tensor_tensor(out=mask1[k][:],
                                                    in0=mem1[k][:], in1=vs,
                                                    op=Alu.is_ge)
                        else:
                            sc = iup.tile([128, NN], F32,
                                          name=f"s1_{t}_{k}", tag="iut")
                            nc.vector.tensor_tensor(out=sc[:],
                                                    in0=mem1[k][:], in1=vs,
                                                    op=Alu.is_ge)
                            nc.vector.tensor_max(out=mask1[k][:],
                                                 in0=mask1[k][:], in1=sc[:])
                        if t < 3:
                            for b in range(NIMG):
                                m1s = mem1[k][:, b * PIX:(b + 1) * PIX]
                                vsv = v1t[:, k * NN + b * PIX:][:, :PIX]
                                sc = iup.tile([128, NN], F32,
                                              name=f"s1b_{t}_{k}_{b}", tag="iut")
                                nc.vector.tensor_tensor(
                                    out=sc[:, :PIX], in0=o1_dense(tl, b),
                                    in1=vsv, op=Alu.mult)
                                nc.vector.tensor_sub(out=m1s, in0=m1s,
                                                     in1=sc[:, :PIX])
                    for b in range(NIMG):
                        o1_make_shifts(tl, b)

            def scanF(ps_list, t, pair):
                for k in range(4):
                    ps = ps_list[k]
                    if t == 0:
                        nc.vector.tensor_copy(out=memf[k][:], in_=ps[:])
                    else:
                        nc.vector.tensor_add(out=memf[k][:], in0=memf[k][:],
                                             in1=ps[:])
                    if t < 3:
                        o3_dst = iup.tile([128, NN], F32,
                                          name=f"o3s_{pair}_{t}_{k}",
                                          tag="iut")[:]
                    else:
                        o3t = ostr.tile([128, NN], F32, name=f"o3_{pair}_{k}",
                                        tag="ost")
                        o3_dst = o3t[:]
                    if vthf_c is not None:
                        nc.vector.tensor_scalar(out=o3_dst, in0=memf[k][:],
                                                scalar1=vthf_c, scalar2=None,
                                                op0=Alu.is_ge)
                        if t == 0:
                            nc.vector.tensor_scalar(out=mask3[k][:],
                                                    in0=memf[k][:],
                                                    scalar1=vthf_c, scalar2=None,
                                                    op0=Alu.is_ge)
                        else:
                            nc.vector.scalar_tensor_tensor(
                                out=mask3[k][:], in0=memf[k][:], scalar=vthf_c,
                                in1=mask3[k][:], op0=Alu.is_ge, op1=Alu.max)
                        if t < 3:
                            nc.vector.scalar_tensor_tensor(
                                out=memf[k][:], in0=o3_dst, scalar=-vthf_c,
                                in1=memf[k][:], op0=Alu.mult, op1=Alu.add)
                    else:
                        vs = vft[:, k * NN:(k + 1) * NN]
                        nc.vector.tensor_tensor(out=o3_dst, in0=memf[k][:],
                                                in1=vs, op=Alu.is_ge)
                        if t == 0:
                            nc.vector.tensor_copy(out=mask3[k][:], in_=o3_dst)
                        else:
                            nc.vector.tensor_max(out=mask3[k][:],
                                                 in0=mask3[k][:], in1=o3_dst)
                        if t < 3:
                            nc.vector.tensor_tensor(out=o3_dst, in0=o3_dst,
                                                    in1=vs, op=Alu.mult)
                            nc.vector.tensor_sub(out=memf[k][:], in0=memf[k][:],
                                                 in1=o3_dst)
                    if t == 3:
                        nc.sync.dma_start(
                            out=O3d[pair][:, k * NN:(k + 1) * NN], in_=o3_dst)
                        iut = iup.tile([128, NN], F32, name=f"iu_{pair}_{k}",
                                       tag="iut")
                        nc.scalar.copy(out=iut[:], in_=ps[:])
                        nc.sync.dma_start(out=IUd[pair, k], in_=iut[:])

            # Startup DMA order: pair-0 x(app 0) first, then w1 cok-chunks
            # in fine sub-chunks (a single DMA stream runs ~57 GB/s, so the
            # largest single transfer gates the first matmul), x(app 1),
            # o1p zero rings (needed by scan1(0) ~+55us), wdr, then w2r
            # cok-chunks (first needed ~+100us).
            # p-state warmup: junk matmuls on zeroed tiles keep the PE's HAM
            # clock gate open through the startup DMA window (~17.5us); the
            # first real conv1 then issues at full 2.4GHz instead of ramping
            # (~3us saved). Sized to end at data-arrival: 48 x 392-col
            # (covers the cold-start ramp + bulk) then 24 x 64-col for a
            # fine-grained tail (overrun costs at most ~56ns each).
            wtmp = st.tile([128, 128], BF16, name="wtmp")
            xtmp = st.tile([128, NN], BF16, name="xtmp")
            nc.vector.memset(wtmp[:], 0.0)
            nc.vector.memset(xtmp[:], 0.0)
            pwarm = pp.tile([128, NN], F32, name="warm", tag="ps1")
            nwarm = 48 + 24
            for i in range(48):
                nc.tensor.matmul(pwarm[:], wtmp[:], xtmp[:], start=(i == 0),
                                 stop=False, skip_group_check=True)
            for i in range(24):
                nc.tensor.matmul(pwarm[:, :64], wtmp[:], xtmp[:, :64],
                                 start=False, stop=(i == 23),
                                 skip_group_check=True)
            wrd = iup.tile([128, NN], F32, name="warmrd", tag="iut")
            nc.scalar.copy(out=wrd[:], in_=pwarm[:])

            xv00 = load_x(0, 0)
            CW1 = 2 * 9 * 128
            for q in range(4):
                nsub = 4 if q == 0 else 1
                for h in range(nsub):
                    sl = slice(q * CW1 + h * CW1 // nsub,
                               q * CW1 + (h + 1) * CW1 // nsub)
                    nc.sync.dma_start(out=w1h[:, sl], in_=W1Hd[:, sl])
                for h in range(nsub):
                    sl = slice(q * CW1 + h * CW1 // nsub,
                               q * CW1 + (h + 1) * CW1 // nsub)
                    nc.sync.dma_start(out=w1l[:, sl], in_=W1Ld[:, sl])
            for k in range(4):
                for par in range(2):
                    nc.sync.dma_start(out=o1p[k][par][:], in_=ZPADd[:])
            xv01 = load_x(0, 1)
            nc.sync.dma_start(out=wdr[:], in_=WDRd[:])
            CW2 = 4 * 9 * 128
            for q in range(4):
                sl = slice(q * CW2, (q + 1) * CW2)
                nc.sync.dma_start(out=w2r[:, sl], in_=W2Rd[:, sl])

            for pair in range(NPAIR):
                xv = {}
                if pair == 0:
                    xv[0], xv[1] = xv00, xv01
                else:
                    xv[0] = load_x(pair, 0)
                    xv[1] = load_x(pair, 1)

                ps1 = {0: [conv1_group(xv[0][0], xv[0][1], k) for k in range(4)]}
                scan1(ps1[0], 0)

                ps1[1] = [conv1_group(xv[1][0], xv[1][1], k) for k in range(4)]
                ps2 = {0: [conv2d_group(xv[0][2], o1_views(0), k)
                           for k in range(4)]}
                xv[2] = load_x(pair, 2)
                scan1(ps1[1], 1)
                scanF(ps2[0], 0, pair)

                ps1[2] = [conv1_group(xv[2][0], xv[2][1], k) for k in range(4)]
                ps2[1] = [conv2d_group(xv[1][2], o1_views(1), k)
                          for k in range(4)]
                xv[3] = load_x(pair, 3)
                scan1(ps1[2], 2)
                scanF(ps2[1], 1, pair)

                ps1[3] = [conv1_group(xv[3][0], xv[3][1], k) for k in range(4)]
                ps2[2] = [conv2d_group(xv[2][2], o1_views(0), k)
                          for k in range(4)]
                xc, xcd = load_xc(pair)
                scan1(ps1[3], 3)
                scanF(ps2[2], 2, pair)

                ps2[3] = [conv2d_group(xv[3][2], o1_views(1), k)
                          for k in range(4)]
                scanF(ps2[3], 3, pair)

                # ANN branch: a = relu(conv1(inp_c)) * mask1 into the parity-0
                # padded tiles (free after conv2d of t=2), then
                # out_c = relu(conv2(a) + convd(inp_c)) * mask3.
                ps_c1 = [conv1_group_ann(xc, k) for k in range(4)]
                for k in range(4):
                    tl = o1p[k][0]
                    sc = iup.tile([128, NN], F32, name=f"ar_{pair}_{k}",
                                  tag="iut")
                    nc.scalar.activation(sc[:], ps_c1[k][:], Act.Relu)
                    for b in range(NIMG):
                        nc.vector.tensor_tensor(
                            out=o1_dense(tl, b),
                            in0=sc[:, b * PIX:(b + 1) * PIX],
                            in1=mask1[k][:, b * PIX:(b + 1) * PIX],
                            op=Alu.mult)
                        o1_make_shifts(tl, b)
                ps_c2 = [conv2d_group(xcd, o1_views(0), k)
                         for k in range(4)]
                for k in range(4):
                    oct_ = ostr.tile([128, NN], F32, name=f"oc_{pair}_{k}",
                                     tag="ost")
                    nc.scalar.activation(oct_[:], ps_c2[k][:], Act.Relu)
                    nc.vector.tensor_tensor(out=oct_[:], in0=oct_[:],
                                            in1=mask3[k][:], op=Alu.mult)
                    for b in range(NIMG):
                        sl = slice(b * PIX, (b + 1) * PIX)
                        nc.sync.dma_start(
                            out=OCd[pair][:, k * NN + b * PIX:][:, :PIX],
                            in_=oct_[:, sl])

    nc.finalize()
    return nc


def _pack_weights(w):
    # cok-major: [128part=cin_low, (cok, cik, tap, 128 cout_low)] so per-cok
    # DMA chunks unlock conv groups incrementally at startup.
    Co, Ci, kh, kw = w.shape
    nchunk = Ci // 128
    a = w.reshape(4, 128, nchunk, 128, kh * kw)  # [cok, co_low, cik, ci_low, tap]
    return np.ascontiguousarray(
        a.transpose(3, 0, 2, 4, 1).reshape(128, 4 * nchunk * kh * kw * 128))


def _vth_const(v):
    v = np.asarray(v, np.float32)
    return float(v.flat[0]) if np.all(v == v.flat[0]) else None


def _vth_rep(v):
    # [512,14,14] -> [128, (chunk, img, pix)] replicated over the image pair
    a = np.asarray(v, np.float32).reshape(4, 128, PIX)
    a = np.broadcast_to(a[:, None, :, :], (4, NIMG, 128, PIX))
    return np.ascontiguousarray(a.transpose(2, 0, 1, 3).reshape(128, 4 * NN))


def _bf_split(a):
    h = a.astype(ml_dtypes.bfloat16)
    l = (a - h.astype(np.float32)).astype(ml_dtypes.bfloat16)
    return np.ascontiguousarray(h), np.ascontiguousarray(l)


def _r_split(a):
    """f32r (11-mantissa-bit) hi/lo split, both stored as fp32 bits."""
    i = a.astype(np.float32).view(np.uint32).astype(np.uint64)
    i = (i + ((i >> 12) & 1) + 0x7FF) & np.uint64(0xFFFFF000)
    h = i.astype(np.uint32).view(np.float32)
    return np.ascontiguousarray(h), np.ascontiguousarray(a - h)


def _planes(x, dup=False):
    """[..., 28, 28] -> [..., 842 or 1248] zero-padded stride-2 parity planes
    (15x15, 15x14, 14x15, 14x14 + tail pad). With dup=True, appends 14-wide
    copies of the even-column planes shifted one column (for kx=2 taps:
    run starts stay 4-byte aligned and 14-row reads are contiguous)."""
    sh = x.shape[:-2]
    xp = np.zeros(sh + (30, 30), np.float32)
    xp[..., 1:29, 1:29] = x
    n = 1248 if dup else 842
    out = np.zeros(sh + (n,), np.float32)
    out[..., 0:225] = xp[..., 0:30:2, 0:30:2].reshape(sh + (225,))
    out[..., 225:435] = xp[..., 0:30:2, 1:29:2].reshape(sh + (210,))
    out[..., 435:645] = xp[..., 1:29:2, 0:30:2].reshape(sh + (210,))
    out[..., 645:841] = xp[..., 1:29:2, 1:29:2].reshape(sh + (196,))
    if dup:
        out[..., 842:1052] = xp[..., 0:30:2, 2:30:2].reshape(sh + (210,))
        out[..., 1052:1248] = xp[..., 1:29:2, 2:30:2].reshape(sh + (196,))
    return out


def kernel(inp_s, inp_u, inp_c, conv1_w, conv2_w, ds_w,
           bn1_gamma, bn1_beta, bn1_mean, bn1_var,
           bn2_gamma, bn2_beta, bn2_mean, bn2_var,
           dsbn_gamma, dsbn_beta, dsbn_mean, dsbn_var,
           vth1, vth2, vth_ds, vth_if):
    global LAST_RESULT
    f32 = lambda x: np.asarray(x, np.float32)
    inp_s, inp_c = f32(inp_s), f32(inp_c)

    def fold(w, gamma, beta, mean, var):
        s = f32(gamma) / np.sqrt(f32(var) + np.float32(EPS))
        return f32(w) * s[:, None, None, None], f32(beta) - f32(mean) * s

    w1, b1 = fold(conv1_w, bn1_gamma, bn1_beta, bn1_mean, bn1_var)
    w2, b2 = fold(conv2_w, bn2_gamma, bn2_beta, bn2_mean, bn2_var)
    wd, bd = fold(ds_w, dsbn_gamma, dsbn_beta, dsbn_mean, dsbn_var)
    b2d = b2 + bd

    vth1_c = _vth_const(vth1)
    vthf_c = _vth_const(vth_if)
    bias1_any = bool(np.any(b1 != 0))
    bias2_any = bool(np.any(b2d != 0))

    cfg = (bias1_any, bias2_any, vth1_c, vthf_c)
    if cfg not in _CACHE:
        _CACHE[cfg] = _build(cfg)
    nc = _CACHE[cfg]

    W1H, W1L = _bf_split(_pack_weights(w1))
    W2R = _pack_weights(w2)
    WDR = _pack_weights(wd)

    T, B = inp_s.shape[:2]
    # planes + bf16 hi/lo split; hi carries the kx=2 dup planes (same bf16
    # rounding, so dup values match the main planes bit-for-bit)
    pl_s = _planes(inp_s, dup=True)
    hi_s = pl_s.astype(ml_dtypes.bfloat16)
    lo_s = (pl_s[..., :842] - hi_s[..., :842].astype(np.float32)) \
        .astype(ml_dtypes.bfloat16)
    xd_s = np.ascontiguousarray(inp_s[..., 0::2, 0::2].reshape(T, B, 256, 196))
    pl_c = _planes(inp_c, dup=True).astype(ml_dtypes.bfloat16)
    xcd_c = np.ascontiguousarray(inp_c[..., 0::2, 0::2].reshape(B, 256, 196))

    def pack_xs(a, f):
        # [T, 4img, 256, f] -> [NPAIR, T, 2cik, 128, NIMG*f]
        r = a.reshape(T, NPAIR, NIMG, 2, 128, f)
        return np.ascontiguousarray(
            r.transpose(1, 0, 3, 4, 2, 5).reshape(NPAIR, T, 2, 128, NIMG * f))

    def pack_xc(a, f):
        # [4img, 256, f] -> [NPAIR, 2cik, 128, NIMG*f]
        r = a.reshape(NPAIR, NIMG, 2, 128, f)
        return np.ascontiguousarray(
            r.transpose(0, 2, 3, 1, 4).reshape(NPAIR, 2, 128, NIMG * f))

    in_maps = []
    for core in range(NCORES):
        b0 = core * BPC
        m = {
            "W1H": W1H, "W1L": W1L, "W2R": W2R, "WDR": WDR,
            "XSH": pack_xs(hi_s[:, b0:b0 + BPC], 1248),
            "XSL": pack_xs(lo_s[:, b0:b0 + BPC], 842),
            "XD": pack_xs(xd_s[:, b0:b0 + BPC], 196),
            "XC": pack_xc(pl_c[b0:b0 + BPC], 1248),
            "ZPAD": np.zeros((128, 1288), np.float32),
            "XCD": pack_xc(xcd_c[b0:b0 + BPC], 196),
        }
        if bias1_any:
            bh, bl = _bf_split(b1.reshape(1, 512))
            m["B1H"], m["B1L"] = bh, bl
        if bias2_any:
            bh, bl = _r_split(b2d.reshape(1, 512))
            m["B2H"], m["B2L"] = bh, bl
            m["ONER"] = np.ones((1, NN), np.float32)
        if vth1_c is None:
            m["VTH1R"] = _vth_rep(vth1)
        if vthf_c is None:
            m["VTHFR"] = _vth_rep(vth_if)
        in_maps.append(m)

    from concourse.bass_utils import run_bass_kernel_spmd
    if TRACE:
        try:
            from trn_agent_boot.trn_boot import _ntff_profile_via_ctypes
            from antenv.axon_hooks import set_axon_ntff_profile_hook
            set_axon_ntff_profile_hook(
                _ntff_profile_via_ctypes('/opt/axon/libaxon_pjrt.so'))
        except Exception:
            pass
    res = run_bass_kernel_spmd(nc, in_maps, core_ids=list(range(NCORES)),
                               trace=TRACE)
    LAST_RESULT = res

    o3 = np.empty((B, 512, 14, 14), np.float32)
    iu = np.empty((B, 512, 14, 14), np.float32)
    oc = np.empty((B, 512, 14, 14), np.float32)
    for core in range(NCORES):
        b0 = core * BPC
        for name, dst in (("O3", o3), ("OC", oc)):
            arr = res.results[core][name].reshape(NPAIR, 128, 4, NIMG, PIX)
            arr = arr.transpose(0, 3, 2, 1, 4).reshape(BPC, 512, 14, 14)
            dst[b0:b0 + BPC] = arr
        arr = res.results[core]["IU"].reshape(NPAIR, 4, 128, NIMG, PIX)
        arr = arr.transpose(0, 3, 1, 2, 4).reshape(BPC, 512, 14, 14)
        iu[b0:b0 + BPC] = arr
    return o3, iu, oc
